# revision 31
# baseline (speedup 1.0000x reference)
"""Trainium2 Bass kernel for nn_L2Accuracy (segment_reduce).

Computes, for pred/target [B=32, N=200000, D=3] and ragged segment
boundaries `indices` [B, 9]:
    err[b, n] = ||pred[b,n] - target[b,n]||_2
    per-(batch, segment) sums of err  (device, 8 NeuronCores)
    segment means + per-type means    (host, O(B*G) scalars)

Production path (v3, raw bass; requires all boundaries % 6250 == 0, which
holds for the uniform-partition inputs): data-parallel over batch, 4
batches/core viewed as one [128, 18750] element grid — partition p owns
the contiguous vertex run [6250p, 6250(p+1)), so every (batch, segment)
is a union of whole partitions and per-segment sums fall out of
per-partition row sums.  The free axis is cut into 10 chunks; per chunk:
    sync  HWDGE q : pred chunk DMA   [128, 1875]  (~1 MB, 7.5 KB rows)
    scalar HWDGE q: target chunk DMA              (2nd queue => 16 SDMA
                    engines engage instead of 5; 343-418 GB/s sustained)
    vector        : tp -= tt          (in-place subtract)
    scalar        : tp = tp^2         (in-place Square)
    vector        : sv = sum over D=3 (3D-view tensor_reduce, axis X)
    scalar        : sv = sqrt(sv), accum_out -> acc[:, chunk]
Hand-rolled semaphores (per-chunk DMA sems to avoid torn reads from
interleaved slice completions; sems cleared by their sole waiting engine
at start since they persist across NEFF executions) skip the Tile
framework's preamble/exit barriers (~10 us).  Host: acc columns ->
per-partition sums -> segment sums (incl. the reference's tail-aliasing
into the next batch's segment 0) -> segment means -> per-type means.

Fallbacks for other boundary patterns: v2 (TileContext version of the
same layout), the bs-block path (boundaries sharing a divisor >= 50),
and a fully general piece/supertile path.
"""

import os
import sys

sys.path.insert(0, "/opt/trn_rl_repo")

import numpy as np

B, N, D = 32, 200000, 3
G, T = 8, 5
NCORES = 8
BPC = B // NCORES          # batches per core (fast path)
EPB = N * D                # elements per batch
FMAX_V = 1600              # max vertices per partition in one piece
STILE_V = 1600             # max vertices per partition in one supertile

_prog_cache = {}


# ---------------------------------------------------------------- host schedule


def _ranges_from_bnd(bnd):
    """9 contiguous vertex ranges partitioning [0, N) for one batch.

    Range r in 0..7 holds vertices with sid == r per the reference's
    searchsorted(bnd[1:], pos, 'right'); range 8 is the tail [bnd[8], N)
    whose vertices alias into the next batch's segment 0.
    """
    starts = [0] + [int(bnd[j]) for j in range(1, G + 1)]
    ends = [int(bnd[j]) for j in range(1, G + 1)] + [N]
    return [(s, max(0, e - s)) for s, e in zip(starts, ends)]


def _pieces_for_range(voff, vcnt):
    """Cover vcnt vertices from voff with [P, F] rects, 3 | F, P <= 128."""
    if vcnt == 0:
        return []
    for P in range(128, 63, -1):
        if vcnt % P == 0 and vcnt // P <= FMAX_V:
            return [(voff, P, 3 * (vcnt // P))]
    pieces, v, left = [], voff, vcnt
    while left > 0:
        P = min(128, left)
        fv = max(1, min(FMAX_V, left // P))
        pieces.append((v, P, 3 * fv))
        v += P * fv
        left -= P * fv
    return pieces


def _build_table(bnds):
    """Piece/supertile schedule for a list of per-batch boundary rows.

    Returns (supertiles, col_map, ncols):
      supertiles: list of (P, [(elem_off, F, col), ...])
      col_map:    col -> (batch_local, range_idx)
    """
    pieces = []          # (batch_local, range_idx, elem_off, P, F)
    for bl, bnd in enumerate(bnds):
        for r, (vs, vc) in enumerate(_ranges_from_bnd(bnd)):
            for (v0, P, F) in _pieces_for_range(vs, vc):
                pieces.append((bl, r, bl * EPB + 3 * v0, P, F))

    supertiles, col_map = [], []
    cur_p, cur_list, cur_fv = None, [], 0
    for (bl, r, eoff, P, F) in pieces:
        col = len(col_map)
        col_map.append((bl, r))
        if cur_p != P or cur_fv + F // 3 > STILE_V:
            if cur_list:
                supertiles.append((cur_p, cur_list))
            cur_p, cur_list, cur_fv = P, [], 0
        cur_list.append((eoff, F, col))
        cur_fv += F // 3
    if cur_list:
        supertiles.append((cur_p, cur_list))
    return supertiles, col_map, len(col_map)


# ------------------------------------------------------- fast block-sum path
#
# When all batches share one boundary vector whose entries divide by a
# block size bs (bs | 800, bs >= 50), each batch is two [125, 2400]-elem
# half-tiles (one contiguous 9.6 KB run per partition -> ~125 DMA packets
# per 1.2 MB DMA instead of per-range shattering), and per-(row, block)
# err sums [125, 2*J2] per batch stream out for host reduceat assembly.


def _fast_bs(bnds):
    import math

    if not all((bnds[i] == bnds[0]).all() for i in range(1, len(bnds))):
        return None
    bs = 800
    for v in bnds[0].tolist():
        bs = math.gcd(bs, int(v))
    return bs if bs >= 50 else None


def _build_program_fast(bs):
    import concourse.bacc as bacc
    import concourse.mybir as mybir
    from concourse.tile import TileContext

    f32 = mybir.dt.float32
    Act = mybir.ActivationFunctionType
    J2 = 800 // bs  # blocks per half-row
    ncols = BPC * 2 * J2

    nc = bacc.Bacc(
        "TRN2", target_bir_lowering=False, debug=False, num_devices=NCORES
    )
    pred_t = nc.dram_tensor("pred", [BPC * EPB], f32, kind="ExternalInput").ap()
    targ_t = nc.dram_tensor("target", [BPC * EPB], f32, kind="ExternalInput").ap()
    out_t = nc.dram_tensor("out", [125, ncols], f32, kind="ExternalOutput").ap()

    with TileContext(nc) as tc:
        with (
            tc.tile_pool(name="io", bufs=4) as io_pool,
            tc.tile_pool(name="work", bufs=3) as w_pool,
            tc.tile_pool(name="stat", bufs=1) as s_pool,
        ):
            eb = s_pool.tile([125, ncols], f32)
            for b in range(BPC):
                for h in range(2):
                    # partition p holds elements [b*EPB + 4800p + 2400h, +2400)
                    tp = io_pool.tile([125, 2400], f32, tag="tp")
                    tt = io_pool.tile([125, 2400], f32, tag="tt")
                    src = pred_t[b * EPB : (b + 1) * EPB].rearrange(
                        "(p f) -> p f", p=125
                    )[:, 2400 * h : 2400 * h + 2400]
                    nc.sync.dma_start(tp[:], src)
                    src = targ_t[b * EPB : (b + 1) * EPB].rearrange(
                        "(p f) -> p f", p=125
                    )[:, 2400 * h : 2400 * h + 2400]
                    nc.sync.dma_start(tt[:], src)
                    diff = w_pool.tile([125, 2400], f32, tag="diff")
                    nc.gpsimd.tensor_tensor(
                        diff[:], tp[:], tt[:], mybir.AluOpType.subtract
                    )
                    nc.scalar.activation(diff[:], diff[:], Act.Square)
                    sv = w_pool.tile([125, 800], f32, tag="sv")
                    nc.vector.tensor_reduce(
                        sv[:],
                        diff[:].rearrange("p (v d) -> p v d", d=3),
                        axis=mybir.AxisListType.X,
                        op=mybir.AluOpType.add,
                    )
                    nc.scalar.activation(sv[:], sv[:], Act.Sqrt)
                    c0 = (b * 2 + h) * J2
                    nc.vector.tensor_reduce(
                        eb[:, c0 : c0 + J2],
                        sv[:].rearrange("p (j v) -> p j v", v=bs),
                        axis=mybir.AxisListType.X,
                        op=mybir.AluOpType.add,
                    )
            nc.sync.dma_start(out_t, eb[:])

    nc.compile()
    return nc


def _fast_host_assemble(core_outs, bnd0, bs):
    """core_outs: per-core [125, BPC*2*J2] block sums -> piece_sums [B, G+1]."""
    J2 = 800 // bs
    nblk = 125 * 2 * J2
    edges = [0] + [int(bnd0[j]) // bs for j in range(1, G + 1)] + [nblk]
    piece_sums = np.zeros((B, G + 1), dtype=np.float64)
    for c, out in enumerate(core_outs):
        out = out.reshape(125, BPC, 2 * J2)
        for bl in range(BPC):
            flat = out[:, bl, :].reshape(-1)  # g = p*(2*J2) + h*J2 + j
            csum = np.concatenate([[0.0], np.cumsum(flat, dtype=np.float64)])
            for r in range(G + 1):
                piece_sums[c * BPC + bl, r] = csum[edges[r + 1]] - csum[edges[r]]
    return piece_sums


# ------------------------------------------------------------ v2 fastest path
#
# When every boundary is a multiple of 6250 (= N/32), the 4 batches a core
# owns form one [128, 18750] element grid (partition p holds the contiguous
# vertex run [6250p, 6250(p+1)) of the core's 800k-vertex stream), and every
# (batch, segment) is a union of whole partitions.  The free axis is cut
# into V2_NCH chunks; per chunk:
#   DMA    : pred / target [128, 18750/V2_NCH]  (1-2 MB contiguous rows)
#   vector : diff = pred - target
#   scalar : diff = diff^2              (in-place)
#   vector : sv   = sum over D=3        (3D-view reduce, axis X)
#   scalar : sv   = sqrt(sv), accum_out -> acc[:, chunk]   (row sums free)
# Host folds acc columns -> per-partition sums -> segment sums.

V2_PART_V = 6250            # vertices per partition (4 batches / 128)
# production config: raw-bass v3 pipeline, two-add d-reduce, geometric
# tail taper (progressively smaller final chunks so their ladders start
# earlier and pipeline across engines instead of draining after the stream)
V2_CFG = dict(v3=True, red2=True,
              chunks=[625, 625, 625, 625, 625, 625, 625, 625, 625,
                      300, 200, 125])


def _build_program_v2(nch=5, chunks=None, split="none", io_bufs=3,
                      inplace=False, sub_engine="vector", red_mode="reduce"):
    import concourse.bacc as bacc
    import concourse.mybir as mybir
    from concourse.tile import TileContext

    f32 = mybir.dt.float32
    Act = mybir.ActivationFunctionType
    VTOT = BPC * EPB // (128 * 3)    # 6250 vertices per partition
    if chunks is None:
        assert VTOT % nch == 0
        chunks = [VTOT // nch] * nch
    assert sum(chunks) == VTOT
    nch = len(chunks)

    nc = bacc.Bacc(
        "TRN2", target_bir_lowering=False, debug=False, num_devices=NCORES
    )
    pred_t = nc.dram_tensor("pred", [BPC * EPB], f32, kind="ExternalInput").ap()
    targ_t = nc.dram_tensor("target", [BPC * EPB], f32, kind="ExternalInput").ap()
    out_t = nc.dram_tensor("out", [128, nch], f32, kind="ExternalOutput").ap()

    pred_v = pred_t.rearrange("(p f) -> p f", p=128)
    targ_v = targ_t.rearrange("(p f) -> p f", p=128)

    def dma_engines(c):
        # (pred_engine, target_engine) for chunk c
        if split == "none":
            return nc.sync, nc.sync
        if split == "2way":
            return nc.sync, nc.scalar
        if split == "3way":
            return [(nc.sync, nc.scalar), (nc.gpsimd, nc.sync),
                    (nc.scalar, nc.gpsimd)][c % 3]
        raise ValueError(split)

    with TileContext(nc) as tc:
        with (
            tc.tile_pool(name="io", bufs=io_bufs) as io_pool,
            tc.tile_pool(name="work", bufs=2) as w_pool,
            tc.tile_pool(name="stat", bufs=1) as s_pool,
        ):
            acc = s_pool.tile([128, nch], f32)
            fo = 0
            for c, vc in enumerate(chunks):
                F = 3 * vc
                FV = vc
                tp = io_pool.tile([128, F], f32, tag="tp")
                tt = io_pool.tile([128, F], f32, tag="tt")
                ep, et = dma_engines(c)
                ep.dma_start(tp[:], pred_v[:, fo : fo + F])
                et.dma_start(tt[:], targ_v[:, fo : fo + F])
                fo += F

                if inplace:
                    diff = tp
                else:
                    diff = w_pool.tile([128, F], f32, tag="diff")
                sub = nc.vector if sub_engine == "vector" else nc.gpsimd
                sub.tensor_tensor(
                    diff[:], tp[:], tt[:], mybir.AluOpType.subtract
                )
                nc.scalar.activation(diff[:], diff[:], Act.Square)

                sv = w_pool.tile([128, FV], f32, tag="sv")
                if red_mode == "reduce":
                    nc.vector.tensor_reduce(
                        sv[:],
                        diff[:].rearrange("p (v d) -> p v d", d=3),
                        axis=mybir.AxisListType.X,
                        op=mybir.AluOpType.add,
                    )
                else:
                    d3 = diff[:].rearrange("p (v d) -> p v d", d=3)
                    eng1 = nc.vector if red_mode == "adds" else nc.gpsimd
                    eng1.tensor_tensor(
                        sv[:], d3[:, :, 0], d3[:, :, 1], mybir.AluOpType.add
                    )
                    eng1.tensor_tensor(
                        sv[:], sv[:], d3[:, :, 2], mybir.AluOpType.add
                    )

                nc.scalar.activation(
                    sv[:], sv[:], Act.Sqrt, accum_out=acc[:, c : c + 1]
                )
            nc.sync.dma_start(out_t, acc[:])

    nc.compile()
    return nc


def _v2_host_assemble(core_outs, bnds):
    """core_outs: per-core [128, V2_NCH] -> piece_sums [B, G+1].

    Partition p of core c holds vertices [6250p, 6250(p+1)) of the core's
    4-batch stream: batch_local = p//32, vertex offset (p%32)*6250.
    """
    piece_sums = np.zeros((B, G + 1), dtype=np.float64)
    for c, out in enumerate(core_outs):
        part = np.asarray(out, dtype=np.float64).sum(axis=1)  # [128]
        for p in range(128):
            bl = p // 32
            b = c * BPC + bl
            v0 = (p % 32) * V2_PART_V
            r = int(np.searchsorted(bnds[b, 1:], v0, side="right"))
            piece_sums[b, r] += part[p]
    return piece_sums


# ----------------------------------------------------------- v3 raw-bass path
#
# Same math and layout as v2, but hand-rolled semaphores instead of
# TileContext, skipping the Tile preamble/exit overhead (~8 us).  Every
# chunk has its own SBUF buffers (no reuse), so all deps are forward RAW:
#   sync  : pred DMAs (q1), final out DMA
#   scalar: target DMAs (q10), Square (in-place), Sqrt+accum
#   vector: subtract (in-place into tp), 3:1 d-reduce
# SBUF/partition: 10*(7.5K tp + 7.5K tt + 2.5K sv) = 175K of 208K.


def _build_program_v3(nch=10, chunks=None, split_out=True, no_gpsimd_drain=True,
                      sub_split=False, red2=False, tt_pair=0, tt_on_sync=0,
                      pt_merge=False):
    import contextlib

    import concourse.bacc as bacc
    import concourse.mybir as mybir

    f32 = mybir.dt.float32
    Act = mybir.ActivationFunctionType
    VTOT = BPC * EPB // (128 * 3)
    if chunks is None:
        assert VTOT % nch == 0
        chunks = [VTOT // nch] * nch
    assert sum(chunks) == VTOT
    nch = len(chunks)

    nc = bacc.Bacc(
        "TRN2", target_bir_lowering=False, debug=False, num_devices=NCORES
    )
    pred_t = nc.dram_tensor("pred", [BPC * EPB], f32, kind="ExternalInput").ap()
    targ_t = nc.dram_tensor("target", [BPC * EPB], f32, kind="ExternalInput").ap()
    out_t = nc.dram_tensor("out", [128, nch], f32, kind="ExternalOutput").ap()
    pred_v = pred_t.rearrange("(p f) -> p f", p=128)
    targ_v = targ_t.rearrange("(p f) -> p f", p=128)
    offs = [3 * sum(chunks[:c]) for c in range(nch)]
    # target DMAs cover pairs of the first tt_pair*2 chunks (fatter packets,
    # fewer ACT issue instructions); remaining chunks get single DMAs
    groups = [(2 * g, 2) for g in range(tt_pair)]
    groups += [(c, 1) for c in range(2 * tt_pair, nch)]
    gidx = {}
    for gi, (c0, n) in enumerate(groups):
        for c in range(c0, c0 + n):
            gidx[c] = gi

    with contextlib.ExitStack() as ctx:
        ps = [ctx.enter_context(nc.semaphore(f"ps{c}")) for c in range(nch)]
        if pt_merge:
            assert tt_pair == 0 and not sub_split
            ts = ps          # both DMAs of a chunk inc one sem; wait >= 32
        else:
            ts = [
                ctx.enter_context(nc.semaphore(f"ts{g}"))
                for g in range(len(groups))
            ]
        subs = ctx.enter_context(nc.semaphore("subs"))
        gsubs = ctx.enter_context(nc.semaphore("gsubs"))
        sqs = ctx.enter_context(nc.semaphore("sqs"))
        reds = ctx.enter_context(nc.semaphore("reds"))
        os_ = ctx.enter_context(nc.semaphore("os"))
        tp = [
            ctx.enter_context(nc.sbuf_tensor(f"tp{c}", [128, 3 * v], f32))
            for c, v in enumerate(chunks)
        ]
        ttg = [
            ctx.enter_context(
                nc.sbuf_tensor(
                    f"tt{g}",
                    [128, 3 * sum(chunks[c0 : c0 + n])],
                    f32,
                )
            )
            for g, (c0, n) in enumerate(groups)
        ]

        def tt_slice(c):
            g = gidx[c]
            c0 = groups[g][0]
            lo = 3 * sum(chunks[c0:c])
            return ttg[g].ap()[:, lo : lo + 3 * chunks[c]]
        sv = [
            ctx.enter_context(nc.sbuf_tensor(f"sv{c}", [128, v], f32))
            for c, v in enumerate(chunks)
        ]
        acc = ctx.enter_context(nc.sbuf_tensor("acc", [128, nch], f32))

        def emit_red(vector, c):
            d3 = tp[c].ap().rearrange("p (v d) -> p v d", d=3)
            if red2:
                vector.tensor_tensor(
                    sv[c].ap(), d3[:, :, 0], d3[:, :, 1], mybir.AluOpType.add
                )
                vector.tensor_tensor(
                    sv[c].ap(), sv[c].ap(), d3[:, :, 2], mybir.AluOpType.add
                ).then_inc(reds, 1)
            else:
                vector.tensor_reduce(
                    sv[c].ap(),
                    d3,
                    axis=mybir.AxisListType.X,
                    op=mybir.AluOpType.add,
                ).then_inc(reds, 1)

        # sems persist across NEFF executions; each is cleared at the top of
        # its sole waiting engine's section, well before the first inc can
        # arrive (first DMA completion is several us after the rendezvous)
        with nc.Block(no_gpsimd_drain=no_gpsimd_drain) as block:

            @block.sync
            def _(sync):
                sync.sem_clear(os_)
                for c in range(nch):
                    sync.dma_start(
                        tp[c].ap(), pred_v[:, offs[c] : offs[c] + 3 * chunks[c]]
                    ).then_inc(ps[c], 16)
                    if c < tt_on_sync:
                        g = gidx[c]
                        c0, n = groups[g]
                        assert n == 1
                        sync.dma_start(
                            ttg[g].ap(),
                            targ_v[:, offs[c0] : offs[c0] + 3 * chunks[c0]],
                        ).then_inc(ts[g], 16)
                sync.wait_ge(os_, 16)

            @block.scalar
            def _(scalar):
                scalar.sem_clear(subs)
                scalar.sem_clear(reds)
                for g, (c0, n) in enumerate(groups):
                    if c0 < tt_on_sync:
                        continue
                    flen = 3 * sum(chunks[c0 : c0 + n])
                    scalar.dma_start(
                        ttg[g].ap(), targ_v[:, offs[c0] : offs[c0] + flen]
                    ).then_inc(ts[g], 16)
                for c in range(nch):
                    if sub_split and c % 2 == 1:
                        scalar.wait_ge(gsubs, (c + 1) // 2)
                    else:
                        scalar.wait_ge(subs, c // 2 + 1 if sub_split else c + 1)
                    scalar.activation(tp[c].ap(), tp[c].ap(), Act.Square).then_inc(
                        sqs, 1
                    )
                    if c >= 1:
                        scalar.wait_ge(reds, c)
                        scalar.activation(
                            sv[c - 1].ap(),
                            sv[c - 1].ap(),
                            Act.Sqrt,
                            accum_out=acc.ap()[:, c - 1 : c],
                        )
                scalar.wait_ge(reds, nch)
                scalar.activation(
                    sv[nch - 1].ap(),
                    sv[nch - 1].ap(),
                    Act.Sqrt,
                    accum_out=acc.ap()[:, nch - 1 : nch],
                )
                # the ACT sequencer runs ahead of its datapath: without a
                # drain the out DMA's SBUF read races the final accum write
                scalar.drain()
                scalar.dma_start(out_t, acc.ap()).then_inc(os_, 16)

            if sub_split:

                @block.gpsimd
                def _(gpsimd):
                    for c in range(1, nch, 2):
                        gpsimd.sem_clear(ps[c])
                    for c in range(1, nch, 2):
                        gpsimd.wait_ge(ps[c], 16)
                        gpsimd.wait_ge(ts[gidx[c]], 16)
                        gpsimd.tensor_tensor(
                            tp[c].ap(), tp[c].ap(), tt_slice(c),
                            mybir.AluOpType.subtract,
                        ).then_inc(gsubs, 1)

            @block.vector
            def _(vector):
                vec_subs = range(0, nch, 2) if sub_split else range(nch)
                for c in vec_subs:
                    vector.sem_clear(ps[c])
                if not pt_merge:
                    for g in range(len(groups)):
                        vector.sem_clear(ts[g])
                vector.sem_clear(sqs)
                for c in range(nch):
                    if c in vec_subs:
                        if pt_merge:
                            vector.wait_ge(ps[c], 32)
                        else:
                            vector.wait_ge(ps[c], 16)
                            vector.wait_ge(ts[gidx[c]], 16)
                        vector.tensor_tensor(
                            tp[c].ap(), tp[c].ap(), tt_slice(c),
                            mybir.AluOpType.subtract,
                        ).then_inc(subs, 1)
                    if c >= 1:
                        vector.wait_ge(sqs, c)
                        emit_red(vector, c - 1)
                vector.wait_ge(sqs, nch)
                emit_red(vector, nch - 1)

    nc.compile()
    return nc


# ---------------------------------------------------------------- device build


def _build_program(nb, supertiles, ncols, num_devices):
    import concourse.bacc as bacc
    import concourse.mybir as mybir
    from concourse.tile import TileContext

    f32 = mybir.dt.float32
    Act = mybir.ActivationFunctionType

    nc = bacc.Bacc(
        "TRN2", target_bir_lowering=False, debug=False, num_devices=num_devices
    )
    pred_t = nc.dram_tensor("pred", [nb * EPB], f32, kind="ExternalInput").ap()
    targ_t = nc.dram_tensor("target", [nb * EPB], f32, kind="ExternalInput").ap()
    out_t = nc.dram_tensor("out", [1, ncols], f32, kind="ExternalOutput").ap()

    with TileContext(nc) as tc:
        with (
            tc.tile_pool(name="io", bufs=2) as io_pool,
            tc.tile_pool(name="work", bufs=2) as w_pool,
            tc.tile_pool(name="stat", bufs=1) as s_pool,
            tc.tile_pool(name="psum", bufs=1, space="PSUM") as p_pool,
        ):
            acc = s_pool.tile([128, ncols], f32)
            ones = s_pool.tile([128, 1], f32)
            nc.gpsimd.memset(acc[:], 0.0)
            nc.gpsimd.memset(ones[:], 1.0)

            for (P, plist) in supertiles:
                ftot = sum(F for (_, F, _) in plist)
                vtot = ftot // 3
                tp = io_pool.tile([P, ftot], f32, tag="tp")
                tt = io_pool.tile([P, ftot], f32, tag="tt")
                fo = 0
                for (eoff, F, _) in plist:
                    src = pred_t[eoff : eoff + P * F].rearrange("(p f) -> p f", p=P)
                    nc.sync.dma_start(tp[:, fo : fo + F], src)
                    src = targ_t[eoff : eoff + P * F].rearrange("(p f) -> p f", p=P)
                    nc.sync.dma_start(tt[:, fo : fo + F], src)
                    fo += F
                diff = w_pool.tile([P, ftot], f32, tag="diff")
                nc.gpsimd.tensor_tensor(
                    diff[:], tp[:], tt[:], mybir.AluOpType.subtract
                )
                nc.scalar.activation(diff[:], diff[:], Act.Square)
                sv = w_pool.tile([P, vtot], f32, tag="sv")
                nc.vector.tensor_reduce(
                    sv[:],
                    diff[:].rearrange("p (v d) -> p v d", d=3),
                    axis=mybir.AxisListType.X,
                    op=mybir.AluOpType.add,
                )
                vo = 0
                for (_, F, col) in plist:
                    fv = F // 3
                    nc.scalar.activation(
                        sv[:, vo : vo + fv],
                        sv[:, vo : vo + fv],
                        Act.Sqrt,
                        accum_out=acc[:P, col : col + 1],
                    )
                    vo += fv

            outs = s_pool.tile([1, ncols], f32)
            for c0 in range(0, ncols, 512):
                c1 = min(ncols, c0 + 512)
                ps = p_pool.tile([1, c1 - c0], f32, tag="ps")
                nc.tensor.matmul(
                    ps[:], ones[:], acc[:, c0:c1], start=True, stop=True
                )
                nc.vector.tensor_copy(outs[:, c0:c1], ps[:])
            nc.sync.dma_start(out_t, outs[:])

    nc.compile()
    return nc


def _get_program(nb, bnds_key, supertiles, ncols, num_devices):
    key = (nb, bnds_key, num_devices)
    if key not in _prog_cache:
        _prog_cache[key] = _build_program(nb, supertiles, ncols, num_devices)
    return _prog_cache[key]


# ---------------------------------------------------------------- entry point

TRACE = False
LAST_RESULTS = None


def kernel(pred, target, indices, indices_type):
    global LAST_RESULTS
    from concourse.bass_utils import run_bass_kernel_spmd

    pred = np.asarray(pred, dtype=np.float32)
    target = np.asarray(target, dtype=np.float32)
    bnds = np.asarray(indices).astype(np.int64)
    itype = np.asarray(indices_type, dtype=np.float32)

    if (bnds % V2_PART_V == 0).all() and bnds.shape == (B, G + 1):
        import json as _json

        cfg = dict(V2_CFG)
        use_v3 = cfg.pop("v3", False)
        key = ("v3" if use_v3 else "v2", _json.dumps(cfg, sort_keys=True))
        if key not in _prog_cache:
            if use_v3:
                _prog_cache[key] = _build_program_v3(
                    nch=cfg.get("nch", 10),
                    chunks=cfg.get("chunks"),
                    split_out=cfg.get("split_out", False),
                    no_gpsimd_drain=cfg.get("no_gpsimd_drain", True),
                    sub_split=cfg.get("sub_split", False),
                    red2=cfg.get("red2", False),
                    tt_pair=cfg.get("tt_pair", 0),
                    tt_on_sync=cfg.get("tt_on_sync", 0),
                    pt_merge=cfg.get("pt_merge", False),
                )
            else:
                _prog_cache[key] = _build_program_v2(**cfg)
        nc = _prog_cache[key]
        in_maps = [
            {
                "pred": np.ascontiguousarray(
                    pred[c * BPC : (c + 1) * BPC]
                ).reshape(-1),
                "target": np.ascontiguousarray(
                    target[c * BPC : (c + 1) * BPC]
                ).reshape(-1),
            }
            for c in range(NCORES)
        ]
        res = run_bass_kernel_spmd(nc, in_maps, list(range(NCORES)), trace=TRACE)
        LAST_RESULTS = res
        core_outs = [np.asarray(res.results[c]["out"]) for c in range(NCORES)]
        piece_sums = _v2_host_assemble(core_outs, bnds)
        return _host_finish(piece_sums, bnds, itype)

    bs = _fast_bs(bnds)
    if bs is not None:
        key = ("fast", bs)
        if key not in _prog_cache:
            _prog_cache[key] = _build_program_fast(bs)
        nc = _prog_cache[key]
        in_maps = [
            {
                "pred": np.ascontiguousarray(
                    pred[c * BPC : (c + 1) * BPC]
                ).reshape(-1),
                "target": np.ascontiguousarray(
                    target[c * BPC : (c + 1) * BPC]
                ).reshape(-1),
            }
            for c in range(NCORES)
        ]
        res = run_bass_kernel_spmd(nc, in_maps, list(range(NCORES)), trace=TRACE)
        LAST_RESULTS = res
        core_outs = [np.asarray(res.results[c]["out"]) for c in range(NCORES)]
        piece_sums = _fast_host_assemble(core_outs, bnds[0], bs)
        return _host_finish(piece_sums, bnds, itype)

    tables = [_build_table(bnds[c * BPC : (c + 1) * BPC]) for c in range(NCORES)]
    uniform = all(t == tables[0] for t in tables[1:])

    if uniform:
        supertiles, col_map, ncols = tables[0]
        nc = _get_program(
            BPC, tuple(bnds[:BPC].ravel().tolist()), supertiles, ncols, NCORES
        )
        in_maps = [
            {
                "pred": np.ascontiguousarray(
                    pred[c * BPC : (c + 1) * BPC]
                ).reshape(-1),
                "target": np.ascontiguousarray(
                    target[c * BPC : (c + 1) * BPC]
                ).reshape(-1),
            }
            for c in range(NCORES)
        ]
        res = run_bass_kernel_spmd(
            nc, in_maps, list(range(NCORES)), trace=TRACE
        )
        LAST_RESULTS = res
        core_outs = [np.asarray(res.results[c]["out"]).ravel() for c in range(NCORES)]
        piece_sums = np.zeros((B, G + 1), dtype=np.float64)
        for c in range(NCORES):
            for col, (bl, r) in enumerate(col_map):
                piece_sums[c * BPC + bl, r] += float(core_outs[c][col])
    else:
        supertiles, col_map, ncols = _build_table(bnds)
        nc = _get_program(B, tuple(bnds.ravel().tolist()), supertiles, ncols, 1)
        in_maps = [{"pred": pred.reshape(-1), "target": target.reshape(-1)}]
        res = run_bass_kernel_spmd(nc, in_maps, [0], trace=TRACE)
        LAST_RESULTS = res
        out0 = np.asarray(res.results[0]["out"]).ravel()
        piece_sums = np.zeros((B, G + 1), dtype=np.float64)
        for col, (bl, r) in enumerate(col_map):
            piece_sums[bl, r] += float(out0[col])

    return _host_finish(piece_sums, bnds, itype)


def _host_finish(piece_sums, bnds, itype):
    # ---- host: ragged segment means + per-type means (reference semantics)
    seg_sum = np.zeros(B * G, dtype=np.float64)
    for b in range(B):
        for s in range(G):
            seg_sum[b * G + s] += piece_sums[b, s]
        fid = (b + 1) * G  # tail [bnd[8], N): sid == 8 aliases to flat (b+1)*G
        if fid < B * G:
            seg_sum[fid] += piece_sums[b, G]

    counts = (bnds[:, 1:] - bnds[:, :-1]).reshape(-1).astype(np.float64)
    with np.errstate(divide="ignore", invalid="ignore"):
        seg_mean = seg_sum / counts

    type_id = np.argmax(itype, axis=-1).reshape(-1)
    t_sum = np.zeros(T, dtype=np.float64)
    t_cnt = np.zeros(T, dtype=np.float64)
    for i in range(B * G):
        t_sum[type_id[i]] += seg_mean[i]
        t_cnt[type_id[i]] += 1.0
    with np.errstate(divide="ignore", invalid="ignore"):
        out = np.where(t_cnt > 0, t_sum / np.maximum(t_cnt, 1.0), 0.0)
    return out.astype(np.float32)

